# revision 22
# baseline (speedup 1.0000x reference)
"""GSA (global self-attention / linear attention) Bass kernel for TRN2.

Problem: img[8,256,128,128] -> qkv 1x1-conv -> softmax(k, axis=tokens) ->
context = k_sm @ v^T (per head, 64x64) -> content = ctx^T @ q -> out 1x1-conv.

Strategy (per core, pure data-parallel over batch; 8 batches -> 8 cores).
Everything after the softmax is linear in img, so the whole module collapses
into one 256x256 projection once the per-head context matrices are known:

  Pass A: stream 256-token macro-tiles; k = w_k @ img (fp16 matmuls),
          ek = exp(k - 2) on ScalarE (fp16 out).  Accumulate in PSUM:
            G^T[c,d] += img16T[n,c]^T-contracted ek[n,d]   (fp16)
            S[d]     += ones^T ek[n,d]                     (fp16)
          G^T comes out c-major, exactly what the fold consumes.
          (fp8 DoubleRow was measured on HW to accumulate at reduced
          precision: 2e-2 rel error on G -- unusable for the token sums.)
  Fold:   ctx^T = w_v @ G^T (per-head block-diag select), W_eff^T =
          (ctx^T-diag/S) @ w_out^T, W_comb^T[c,o] = sum_d w_q[d,c] W_eff^T[d,o].
  Pass B: out = W_comb @ img + b, fp16 matmuls, fp16 output (host casts f32).

dtypes: img/weights fp16 in pass A/B projections (5e-4), ek/img fp8e4 in the
token-contracted G/S accumulations (errors average over 16k tokens; measured
end-to-end max-rel ~6e-3 vs the 2e-2 gate). DMA: 10MB in + 8MB out per core.
"""
import numpy as np
import ml_dtypes

HEADS, DK = 8, 64
B, C, X, Y = 8, 256, 128, 128
N_TOK = X * Y          # 16384
DH = HEADS * DK        # 512
N_CORES = 8

TA = 128               # pass A token tile (partition dim of k)
MAC = 2 * TA           # pass A macro-tile (DoubleRow pairs two 128-tok tiles)
TB = 512               # pass B token tile

NP_F8 = ml_dtypes.float8_e4m3
EXP_BIAS = -2.0        # ek = exp(k - 2): keeps exp under fp8e4 max (240)


def _build_program(n_tok=N_TOK, tb=TB, debug=False, repeat=1, kproj="f16",
                   ek_bufs=3, kps_bufs=2, pb_bufs=4, pso_bufs=3,
                   img16_chunk=2048, img8t_chunk=8, out_alt=1):
    from contextlib import ExitStack
    import concourse.bacc as bacc
    import concourse.mybir as mybir
    import concourse.tile as tile

    F32 = mybir.dt.float32
    F32R = mybir.dt.float32r
    F16 = mybir.dt.float16
    F8 = mybir.dt.float8e4
    AF = mybir.ActivationFunctionType
    DR = mybir.MatmulPerfMode.DoubleRow

    nmac = n_tok // MAC
    ntb = n_tok // tb

    nc = bacc.Bacc("TRN2", debug=False, num_devices=N_CORES)
    img16_d = nc.dram_tensor("img16", [C, n_tok], F16, kind="ExternalInput").ap() \
        .rearrange("(c2 p) n -> p c2 n", p=128)
    # host pre-packs img16T as [p, nmac, j, c] (token-major, partition-contig)
    img16t_d = nc.dram_tensor("img16T", [128, nmac * 2 * C], F16, kind="ExternalInput").ap()
    wk_d = nc.dram_tensor("w_kT16", [C, DH], F16, kind="ExternalInput").ap() \
        .rearrange("(c2 p) d -> p c2 d", p=128)
    wk8_d = None
    if kproj == "c8":
        wk8_d = nc.dram_tensor("w_kT8hl", [2, C, DH], F8, kind="ExternalInput").ap() \
            .rearrange("hl (c2 p) d -> p hl c2 d", p=128)
        img8_d = nc.dram_tensor("img8", [C, n_tok], F8, kind="ExternalInput").ap() \
            .rearrange("(c2 p) n -> p c2 n", p=128)
    wvT_d = nc.dram_tensor("w_vT", [C, DH], F32R, kind="ExternalInput").ap() \
        .rearrange("(c2 p) e -> p c2 e", p=128)
    wq_d = nc.dram_tensor("w_q", [DH, C], F32R, kind="ExternalInput").ap() \
        .rearrange("(d4 p) c -> p d4 c", p=128)
    wo_d = nc.dram_tensor("w_outT", [DH, C], F32R, kind="ExternalInput").ap() \
        .rearrange("(e4 p) o -> p e4 o", p=128)
    b_d = nc.dram_tensor("b_out", [C], F32, kind="ExternalInput").ap() \
        .rearrange("(o2 p) -> p o2", p=128)
    out_d = nc.dram_tensor("out", [C, n_tok], F16, kind="ExternalOutput").ap() \
        .rearrange("(o2 p) n -> p o2 n", p=128)
    s_scratch = nc.dram_tensor("s_scratch", [DH], F32).ap()
    dbg = {}
    if debug:
        for name, shape in [("d_gt", [128, 2, DH]), ("d_s", [128, 4]),
                            ("d_wcomb", [128, 2, C]), ("d_ek0", [128, 2, DH])]:
            dbg[name] = nc.dram_tensor(name, shape, F32, kind="ExternalOutput").ap()

    def emit(tc, ctx):
        persist = ctx.enter_context(tc.tile_pool(name="persist", bufs=1))
        small = ctx.enter_context(tc.tile_pool(name="small", bufs=1))
        acc_ctx = ctx.enter_context(ExitStack())
        psacc = acc_ctx.enter_context(tc.tile_pool(name="psacc", bufs=1, space="PSUM"))

        img16_sb = persist.tile([128, 2, n_tok], F16)
        img16t_sb = persist.tile([128, nmac, 2, C], F16)
        wk_sb = persist.tile([128, 2, DH], F16)
        if kproj == "c8":
            wk8_sb = persist.tile([128, 2, 2, DH], F8)
            img8_sb = persist.tile([128, 2, n_tok], F8)
        wvT_sb = persist.tile([128, 2, DH], F32R)
        wq_sb = persist.tile([128, 4, C], F32R)
        woT_sb = persist.tile([128, 4, C], F32R)
        b_sb = persist.tile([128, 2], F32)
        ones16_sb = persist.tile([128, 1], F16)
        wcomb16_sb = persist.tile([128, 2, C], F16)
        weff_sb = persist.tile([128, 4, C], F32R)

        nc.sync.dma_start(out=wk_sb, in_=wk_d)
        nc.sync.dma_start(out=wvT_sb, in_=wvT_d)
        nc.sync.dma_start(out=wq_sb, in_=wq_d)
        nc.sync.dma_start(out=woT_sb, in_=wo_d)
        nc.sync.dma_start(out=b_sb, in_=b_d)
        if kproj == "c8":
            nc.sync.dma_start(out=wk8_sb, in_=wk8_d)
        nc.vector.memset(ones16_sb, 1.0)
        ebias_sb = persist.tile([128, 1], F32)
        nc.vector.memset(ebias_sb, EXP_BIAS)

        # stream inputs in chunks so pass A can start immediately
        nc.sync.dma_start(out=img16t_sb[:, 0:img8t_chunk],
                          in_=img16t_d.rearrange("p (m x) -> p m x", m=nmac)[:, 0:img8t_chunk])
        for j in range(n_tok // img16_chunk):
            sl = slice(j * img16_chunk, (j + 1) * img16_chunk)
            nc.sync.dma_start(out=img16_sb[:, :, sl], in_=img16_d[:, :, sl])
            if kproj == "c8":
                nc.sync.dma_start(out=img8_sb[:, :, sl], in_=img8_d[:, :, sl])
        for m0 in range(img8t_chunk, nmac, img8t_chunk):
            nc.sync.dma_start(
                out=img16t_sb[:, m0:m0 + img8t_chunk],
                in_=img16t_d.rearrange("p (m x) -> p m x", m=nmac)[:, m0:m0 + img8t_chunk])

        # persistent PSUM accumulators: 2 G^T chains (2 banks) + S (1 bank)
        gt_ps = psacc.tile([128, 2, DH], F32)      # [c', c2, d]
        s_ps = psacc.tile([1, DH], F32)

        # ---------------- PASS A (macro-tiles of 2x128 tokens) ----------------
        with ExitStack() as actx:
            pa = actx.enter_context(tc.tile_pool(name="pa", bufs=ek_bufs))
            psk = actx.enter_context(tc.tile_pool(name="psk", bufs=kps_bufs, space="PSUM"))

            def kproj_emit(m, k_ps):
                for j in range(2):
                    slj = slice(m * MAC + j * TA, m * MAC + (j + 1) * TA)
                    if kproj == "c8":
                        for hl in range(2):
                            for dh_ in range(2):
                                dsl = slice(dh_ * 256, (dh_ + 1) * 256)
                                nc.tensor.matmul(
                                    k_ps[:, j, dsl],
                                    lhsT=img8_sb[:, :, slj], rhs=wk8_sb[:, hl, :, dsl],
                                    start=(hl == 0), stop=(hl == 1), perf_mode=DR)
                    else:
                        for c2 in range(2):
                            nc.tensor.matmul(
                                k_ps[:, j, :], lhsT=img16_sb[:, c2, slj],
                                rhs=wk_sb[:, c2, :],
                                start=(c2 == 0), stop=(c2 == 1))

            def gs_emit(m, ek):
                last = m == nmac - 1
                for j in range(2):
                    for c2 in range(2):
                        nc.tensor.matmul(
                            gt_ps[:, c2, :],
                            lhsT=img16t_sb[:, m, j, c2 * 128:(c2 + 1) * 128],
                            rhs=ek[:, j, :],
                            start=(m == 0 and j == 0), stop=(last and j == 1),
                            skip_group_check=True)
                    nc.tensor.matmul(
                        s_ps, lhsT=ones16_sb, rhs=ek[:, j, :],
                        start=(m == 0 and j == 0), stop=(last and j == 1),
                        skip_group_check=True)

            prev = None
            for m in range(nmac):
                k_ps = psk.tile([128, 2, DH], F32, tag="kps")
                kproj_emit(m, k_ps)
                ek = pa.tile([128, 2, DH], F16, tag="ek")
                nc.scalar.activation(out=ek, in_=k_ps, func=AF.Exp, bias=ebias_sb)
                if debug and m == 0:
                    ek32 = pa.tile([128, 2, DH], F32, tag="ek32")
                    nc.vector.tensor_copy(out=ek32, in_=ek)
                    nc.sync.dma_start(out=dbg["d_ek0"], in_=ek32)
                if prev is not None:
                    gs_emit(*prev)
                prev = (m, ek)
            gs_emit(*prev)

        # ---- FOLD ----
        # 1/S: PSUM [1,512] -> DRAM -> [128,4] column layout
        s_sb = small.tile([1, DH], F32)
        nc.vector.tensor_copy(out=s_sb, in_=s_ps)
        nc.sync.dma_start(out=s_scratch, in_=s_sb)
        scol = small.tile([128, 4], F32)
        nc.sync.dma_start(out=scol, in_=s_scratch.rearrange("(f p) -> p f", p=128))
        rs = small.tile([128, 4], F32)
        nc.vector.reciprocal(out=rs, in_=scol)

        gt_sb = small.tile([128, 2, DH], F32R)  # [c', c2, d]
        for c2 in range(2):
            nc.vector.tensor_copy(out=gt_sb[:, c2, :], in_=gt_ps[:, c2, :])
        acc_ctx.close()  # free G/S banks

        with ExitStack() as wctx:
            psw = wctx.enter_context(tc.tile_pool(name="psw", bufs=1, space="PSUM"))

            if debug:
                gt32 = small.tile([128, 2, DH], F32)
                nc.vector.tensor_copy(out=gt32, in_=gt_sb.bitcast(F32))
                nc.sync.dma_start(out=dbg["d_gt"], in_=gt32)
                nc.sync.dma_start(out=dbg["d_s"], in_=rs)

            # ctx^T[e,d] = sum_c w_v[e,c] G^T[c,d]
            ctxT_ps = psw.tile([128, 4, DH], F32)   # [e', pk, d]
            for pk in range(4):
                for c2 in range(2):
                    nc.tensor.matmul(
                        ctxT_ps[:, pk, :],
                        lhsT=wvT_sb[:, c2, pk * 128:(pk + 1) * 128],
                        rhs=gt_sb[:, c2, :],
                        start=(c2 == 0), stop=(c2 == 1))
            # block-diagonal select (per-head 64x64), zero cross-head terms
            ctxd_sb = small.tile([128, 4, 128], F32R)   # [e', pk, d']
            nc.vector.memset(ctxd_sb.bitcast(F32), 0.0)
            for pk in range(4):
                for blk in range(2):
                    psl = slice(blk * 64, (blk + 1) * 64)
                    dsl = slice(pk * 128 + blk * 64, pk * 128 + (blk + 1) * 64)
                    nc.vector.tensor_copy(
                        out=ctxd_sb[psl, pk, blk * 64:(blk + 1) * 64],
                        in_=ctxT_ps[psl, pk, dsl])
            # W_eff^T[d,o] = (1/S_d) sum_e ctx^T[e,d] w_out^T[e,o]
            weff_ps = psw.tile([128, 4, C], F32)
            for pk in range(4):
                nc.tensor.matmul(weff_ps[:, pk, :], lhsT=ctxd_sb[:, pk, :],
                                 rhs=woT_sb[:, pk, :], start=True, stop=True)
            for pk in range(4):
                nc.vector.tensor_scalar_mul(
                    out=weff_sb[:, pk, :], in0=weff_ps[:, pk, :],
                    scalar1=rs[:, pk:pk + 1])
            # W_comb^T[c,o] = sum_d w_q[d,c] W_eff^T[d,o]
            wc_ps = psw.tile([128, 2, C], F32)
            for c2 in range(2):
                csl = slice(c2 * 128, (c2 + 1) * 128)
                for d4 in range(4):
                    nc.tensor.matmul(wc_ps[:, c2, :], lhsT=wq_sb[:, d4, csl],
                                     rhs=weff_sb[:, d4, :],
                                     start=(d4 == 0), stop=(d4 == 3))
            for c2 in range(2):
                nc.vector.tensor_copy(out=wcomb16_sb[:, c2, :], in_=wc_ps[:, c2, :])
            if debug:
                wc32 = small.tile([128, 2, C], F32)
                nc.vector.tensor_copy(out=wc32, in_=wcomb16_sb)
                nc.sync.dma_start(out=dbg["d_wcomb"], in_=wc32)

        # ---------------- PASS B: out = W_comb @ img + b ----------------
        with ExitStack() as bctx:
            pb = bctx.enter_context(tc.tile_pool(name="pb", bufs=pb_bufs))
            pso = bctx.enter_context(tc.tile_pool(name="pso", bufs=pso_bufs, space="PSUM"))
            for i in range(ntb):
                sl = slice(i * tb, (i + 1) * tb)
                out_ps = pso.tile([128, 2, tb], F32)
                for o2 in range(2):
                    for c2 in range(2):
                        nc.tensor.matmul(
                            out_ps[:, o2, :],
                            lhsT=wcomb16_sb[:, c2, o2 * 128:(o2 + 1) * 128],
                            rhs=img16_sb[:, c2, sl],
                            start=(c2 == 0), stop=(c2 == 1))
                out_sb = pb.tile([128, 2, tb], F16, tag="o")
                for o2 in range(2):
                    if out_alt and (2 * i + o2) % 2:
                        nc.vector.tensor_scalar_add(out=out_sb[:, o2, :],
                                                    in0=out_ps[:, o2, :],
                                                    scalar1=b_sb[:, o2:o2 + 1])
                    else:
                        nc.scalar.activation(out=out_sb[:, o2, :], in_=out_ps[:, o2, :],
                                             func=AF.Identity,
                                             bias=b_sb[:, o2:o2 + 1])
                nc.sync.dma_start(out=out_d[:, :, sl], in_=out_sb)

    with tile.TileContext(nc) as tc:
        for _rep in range(repeat):
            with ExitStack() as ctx:
                emit(tc, ctx)
            if repeat > 1:
                tc.strict_bb_all_engine_barrier()

    nc.compile()
    return nc


def _prep_inputs(img, w_qkv, w_out, b_out, n_tok=N_TOK, kproj="f16"):
    imgs = np.ascontiguousarray(np.asarray(img).reshape(B, C, n_tok), dtype=np.float32)
    w_qkv = np.asarray(w_qkv, dtype=np.float32)
    w_q = np.ascontiguousarray(w_qkv[0:DH])                       # [512, 256]
    w_kT16 = np.ascontiguousarray(w_qkv[DH:2 * DH].T.astype(np.float16))   # [256, 512]
    w_vT = np.ascontiguousarray(w_qkv[2 * DH:3 * DH].T)           # [256, 512]
    w_outT = np.ascontiguousarray(np.asarray(w_out, dtype=np.float32).T)  # [512, 256]
    b = np.ascontiguousarray(np.asarray(b_out, dtype=np.float32))
    nmac = n_tok // MAC
    maps = []
    extra = {}
    if kproj == "c8":
        w_k = w_qkv[DH:2 * DH]
        hi = w_k.astype(NP_F8)
        lo = (w_k - hi.astype(np.float32)).astype(NP_F8)
        extra["w_kT8hl"] = np.ascontiguousarray(
            np.stack([hi.T, lo.T]))                               # [2, 256, 512]
    for i in range(B):
        img16 = imgs[i].astype(np.float16)                        # [256, n]
        # [p, m, j, c] pack of img^T in fp16
        img16t = np.ascontiguousarray(
            imgs[i].T.astype(np.float16).reshape(nmac, 2, 128, C)
            .transpose(2, 0, 1, 3).reshape(128, nmac * 2 * C))
        m = {"img16": img16, "img16T": img16t, "w_kT16": w_kT16, "w_vT": w_vT,
             "w_q": w_q, "w_outT": w_outT, "b_out": b, **extra}
        if kproj == "c8":
            m["img8"] = imgs[i].astype(NP_F8)
        maps.append(m)
    return maps


class _Exec:
    """Compile once, execute many times on the 8 cores via PJRT/shard_map."""

    def __init__(self, nc):
        import jax
        import concourse.mybir as mybir
        from jax.experimental.shard_map import shard_map
        from jax.sharding import Mesh, PartitionSpec, NamedSharding
        from concourse.bass2jax import _bass_exec_p, install_neuronx_cc_hook, partition_id_tensor

        install_neuronx_cc_hook()
        self.jax = jax
        in_names, out_names, out_avals = [], [], []
        partition_name = nc.partition_id_tensor.name if nc.partition_id_tensor else None
        for alloc in nc.m.functions[0].allocations:
            if not isinstance(alloc, mybir.MemoryLocationSet):
                continue
            name = alloc.memorylocations[0].name
            if alloc.kind == "ExternalInput":
                if name != partition_name:
                    in_names.append(name)
            elif alloc.kind == "ExternalOutput":
                out_names.append(name)
                out_avals.append(jax.core.ShapedArray(
                    tuple(alloc.tensor_shape), mybir.dt.np(alloc.dtype)))
        self.in_names, self.out_names, self.out_avals = in_names, out_names, out_avals
        n_params = len(in_names)
        all_in_names = in_names + out_names
        if partition_name is not None:
            all_in_names.append(partition_name)

        def _body(*args):
            operands = list(args)
            if partition_name is not None:
                operands.append(partition_id_tensor())
            return tuple(_bass_exec_p.bind(
                *operands,
                out_avals=tuple(out_avals),
                in_names=tuple(all_in_names),
                out_names=tuple(out_names),
                lowering_input_output_aliases=(),
                sim_require_finite=True,
                sim_require_nnan=True,
                nc=nc,
            ))

        devices = jax.devices()[:N_CORES]
        mesh = Mesh(np.asarray(devices), ("core",))
        self._body = _body
        self.mesh = mesh
        self.sharding = NamedSharding(mesh, PartitionSpec("core"))
        n_ops = n_params + len(out_names)
        self.fn = jax.jit(
            shard_map(_body, mesh=mesh,
                      in_specs=(PartitionSpec("core"),) * n_ops,
                      out_specs=(PartitionSpec("core"),) * len(out_names),
                      check_rep=False),
            keep_unused=True,
        )
        self.dev_zeros = [
            jax.device_put(np.zeros((N_CORES * a.shape[0], *a.shape[1:]), a.dtype),
                           self.sharding)
            for a in out_avals
        ]

    def stage(self, in_maps):
        concat = [
            np.concatenate([np.asarray(m[name]) for m in in_maps], axis=0)
            for name in self.in_names
        ]
        return [self.jax.device_put(a, self.sharding) for a in concat]

    def run(self, staged):
        outs = self.fn(*staged, *self.dev_zeros)
        self.jax.block_until_ready(outs)
        return outs

    def results(self, outs):
        per_core = []
        for c in range(N_CORES):
            per_core.append({
                name: np.asarray(outs[i]).reshape(N_CORES, *self.out_avals[i].shape)[c]
                for i, name in enumerate(self.out_names)
            })
        return per_core


_CACHE = {}


def _get_exec():
    if "exec" not in _CACHE:
        _CACHE["exec"] = _Exec(_build_program())
    return _CACHE["exec"]


def kernel(img, w_qkv, w_out, b_out):
    ex = _get_exec()
    staged = ex.stage(_prep_inputs(img, w_qkv, w_out, b_out))
    res = ex.results(ex.run(staged))
    out = np.stack([res[i]["out"] for i in range(N_CORES)])
    return out.astype(np.float32).reshape(B, C, X, Y)
